# revision 16
# baseline (speedup 1.0000x reference)
"""Causal self-attention (B=2, T=4096, D=512, H=8) on 8 TRN2 NeuronCores.

Sharding: head/tensor parallel x data parallel. Core c (0..7) handles
batch b = c // 4 and head pair g = c % 4 (heads 2g, 2g+1). Each core
computes, for its batch and its two heads: the QKV projections, causal
flash attention over the full sequence, and a partial output projection
against its 128 columns of w_out. The host sums the four partial
[T, D] outputs per batch and stacks the two batches.

On-chip layout ("transposed flash"): scores are computed as
S^T[k, q] = K^T_tile.T @ Q^T so softmax normalization reduces over the
PSUM partition axis via an appended ones-column on the V stationary
([V | 1]): numerator rows 0..63, denominator row 64 of one accumulator.
The two heads' 64-deep score matmuls run CONCURRENTLY as PE row-tiles
(head A rows 0:64, head B rows 64:128 via auto tile_position) into
adjacent PSUM banks, and a single exp ACTIVATE per k-tile covers both
banks ([128, 2, 512] AP) — half the ScalarE instruction count. exp
carries bias=-4ln2 (numerator/denominator scale cancels) for range
headroom. The kernel is ONE flattened k-tile stream across query
blocks (PV lags 2 tiles; no per-block pipeline drain), with
projections / V tiles / out-projections woven in as deferred jobs.
V is projected directly in [keys, d] orientation (stationary x^T tile,
moving w_v^T both heads) — no PE transposes. Host pre-casts x/weights
to fp16 (halves input DMA, removes on-chip casts); y partials are
DMA'd out fp16 and summed in fp32 on host. Causal masking multiplies
precomputed 0/1 fp16 tiles on the diagonal-straddling corners post-exp;
fully-masked columns are never computed.
"""

import sys
import types
from contextlib import ExitStack

import numpy as np

B, T, D = 2, 4096, 512
H, HD = 8, 64
QB = 512  # query block (columns of S^T tiles)
KT = 128  # key tile (partition rows of S^T tiles)
NQB = T // QB  # 8
NKT = T // KT  # 32
EC = D // 128  # 4 contraction chunks of 128 over the model dim
BSH = 4 * float(np.log(2.0))  # exp bias: P scaled by 2^-4 (cancels in softmax)


def _install_ntff_shim():
    if "antenv.axon_hooks" in sys.modules:
        return
    mod = types.ModuleType("antenv.axon_hooks")
    mod._hook = None
    mod.set_axon_ntff_profile_hook = lambda h: setattr(mod, "_hook", h)
    mod.get_axon_ntff_profile_hook = lambda: mod._hook
    sys.modules["antenv.axon_hooks"] = mod
    try:
        import antenv

        antenv.axon_hooks = mod
    except ImportError:
        pass
    try:
        from trn_agent_boot.trn_boot import _ntff_profile_via_ctypes

        mod._hook = _ntff_profile_via_ctypes("/opt/axon/libaxon_pjrt.so")
    except Exception:
        pass


_NC_CACHE = {}


def _build():
    import concourse.bass as bass
    import concourse.mybir as mybir
    import concourse.tile as tile
    from concourse import bacc

    F32 = mybir.dt.float32
    F16 = mybir.dt.float16
    EXP = mybir.ActivationFunctionType.Exp
    GE = mybir.AluOpType.is_ge

    nc = bacc.Bacc(None, target_bir_lowering=False)
    xT_in = nc.declare_dram_parameter("xT", [D, T], F16, isOutput=False)
    wqT_in = nc.declare_dram_parameter("wqT", [D, 128], F16, isOutput=False)
    wkT_in = nc.declare_dram_parameter("wkT", [D, 128], F16, isOutput=False)
    wvT_in = nc.declare_dram_parameter("wvT", [D, 128], F16, isOutput=False)
    woT_in = nc.declare_dram_parameter("woT", [128, D], F16, isOutput=False)
    y_out = nc.declare_dram_parameter("y", [T, D], F16, isOutput=True)

    with tile.TileContext(nc) as tc, ExitStack() as ctx:
        const = ctx.enter_context(tc.tile_pool(name="const", bufs=1))
        big = ctx.enter_context(tc.tile_pool(name="big", bufs=1))
        sp_ps = ctx.enter_context(tc.tile_pool(name="sp_ps", bufs=2, space="PSUM"))
        acc_ps = ctx.enter_context(tc.tile_pool(name="acc_ps", bufs=3, space="PSUM"))
        y_ps = ctx.enter_context(tc.tile_pool(name="y_ps", bufs=1, space="PSUM"))
        p_sb = ctx.enter_context(tc.tile_pool(name="p_sb", bufs=6))
        a_sb = ctx.enter_context(tc.tile_pool(name="a_sb", bufs=2))
        d_sb = ctx.enter_context(tc.tile_pool(name="d_sb", bufs=2))
        y_sb = ctx.enter_context(tc.tile_pool(name="y_sb", bufs=3))

        # Per-partition exp bias: P scaled by 2^-4 (cancels in softmax).
        bias_t = const.tile([128, 1], F32)
        nc.gpsimd.memset(bias_t[:], -BSH)

        # Warm the scalar engine's exp table early.
        wsrc = const.tile([1, 1], F32)
        nc.vector.memset(wsrc[:], 0.0)
        warm = const.tile([1, 1], F32)
        nc.scalar.activation(warm[:], wsrc[:], EXP, scale=1.0, bias=bias_t[0:1])

        # Causal masks for the 4 diagonal-straddling k-tile offsets.
        # cmask[k, d, q] = 1.0 iff (q - k - d*KT) >= 0, else 0.
        cmask = const.tile([128, QB // KT, QB], F16)
        nc.gpsimd.memset(cmask[:], 1.0)
        for di in range(QB // KT):
            nc.gpsimd.affine_select(
                out=cmask[:, di, :],
                in_=cmask[:, di, :],
                compare_op=GE,
                fill=0.0,
                base=-di * KT,
                pattern=[[1, QB]],
                channel_multiplier=-1,
            )

        # ---- persistent operands (all fp16, DMA'd or computed) ----
        qT_r = big.tile([128, T], F16)  # head A d in rows 0-63, head B 64-127
        kT2 = big.tile([128, T], F16)  # same row split
        xT_r = big.tile([128, EC, T], F16)
        v_t = big.tile([128, NKT, 2, 65], F16)  # [V | 1] per head per key tile
        w_r = const.tile([128, 3, EC, 128], F16)
        wo_r = const.tile([128, D], F16)

        nc.sync.dma_start(w_r[:, 0], wqT_in.rearrange("(c p) d -> p c d", p=128))
        nc.sync.dma_start(w_r[:, 1], wkT_in.rearrange("(c p) d -> p c d", p=128))
        nc.sync.dma_start(w_r[:, 2], wvT_in.rearrange("(c p) d -> p c d", p=128))
        nc.sync.dma_start(wo_r[:], woT_in[:])

        for c in range(EC):
            nc.sync.dma_start(xT_r[:, c, 0:QB], xT_in[bass.ts(c, 128), 0:QB])
        REM = T - QB
        for hx in range(4):
            lo = QB + hx * (REM // 4)
            for c in range(EC):
                nc.sync.dma_start(
                    xT_r[:, c, bass.ds(lo, REM // 4)],
                    xT_in[bass.ts(c, 128), bass.ds(lo, REM // 4)],
                )

        ones_f = const.tile([128, NKT], F32)
        nc.vector.memset(ones_f[:], 1.0)
        for h in range(2):
            nc.vector.tensor_copy(v_t[:, :, h, 64:65], ones_f[:].unsqueeze(2))

        scale = 1.0 / float(np.sqrt(HD))

        def emit_outproj(aT_prev, Jp, sub):
            yp = y_ps.tile([128, D], F32)
            nc.tensor.matmul(
                yp[:], aT_prev[:, bass.ts(sub, 128)], wo_r[:],
                start=True, stop=True,
            )
            ysb = y_sb.tile([128, D], F16)
            nc.vector.tensor_copy(ysb[:], yp[:])
            nc.sync.dma_start(y_out[bass.ds(Jp * QB + sub * 128, 128), :], ysb[:])

        def emit_qkproj(Jc):
            pt = sp_ps.tile([128, 2, QB], F32, tag="sp")
            for c in range(EC):
                for wi in range(2):  # alternate PSUM banks between MMs
                    nc.tensor.matmul(
                        pt[:, wi, :],
                        w_r[:, wi, c],
                        xT_r[:, c, bass.ts(Jc, QB)],
                        start=(c == 0),
                        stop=(c == EC - 1),
                    )
            nc.vector.tensor_copy(qT_r[:, bass.ts(Jc, QB)], pt[:, 0, :])
            nc.vector.tensor_copy(kT2[:, bass.ts(Jc, QB)], pt[:, 1, :])

        def emit_vtiles(t0):
            # Two key tiles' V in [keys, d] orientation.
            pv = sp_ps.tile([128, 2, QB], F32, tag="sp")
            for j in range(2):
                tq = t0 + j
                for c in range(EC):
                    nc.tensor.matmul(
                        pv[:, 0, bass.ts(j, 128)],
                        xT_r[:, c, bass.ts(tq, KT)],
                        w_r[:, 2, c],
                        start=(c == 0),
                        stop=(c == EC - 1),
                    )
            for j in range(2):
                tq = t0 + j
                nc.vector.tensor_copy(
                    v_t[:, tq, :, 0:64],
                    pv[:, 0, bass.ts(j, 128)].rearrange("p (h d) -> p h d", h=2),
                )

        def emit_division(accJ, aT_dst, h):
            drow = d_sb.tile([1, QB], F32, tag="drow")
            nc.vector.tensor_copy(drow[:], accJ[h][64:65, :])
            bc = d_sb.tile([64, QB], F32, tag="bc")
            nc.gpsimd.partition_broadcast(bc[:], drow[:])
            rbc = d_sb.tile([64, QB], F32, tag="rbc")
            nc.vector.reciprocal_approx_fast(out=rbc[:], in_=bc[:])
            nc.vector.tensor_mul(
                aT_dst[bass.ts(h, 64), :], accJ[h][0:64, :], rbc[:]
            )

        units = [(J, t) for J in range(NQB) for t in range(4 * (J + 1))]
        jobs = []
        acc = {}
        aT = {}
        pts = {}

        def emit_pv(i2):
            # PV for unit i2 (both heads); divisions + outproj at J end.
            Jp, t0 = units[i2]
            pt, lo_p = pts.pop(i2)
            last = t0 + 1 == 4 * (Jp + 1)
            for h in range(2):
                nc.tensor.matmul(
                    acc[Jp][h][:, lo_p:QB],
                    v_t[:, t0, h],
                    pt[:, h, lo_p:QB],
                    start=(t0 == 0),
                    stop=last,
                )
            if last:
                # h0 division completes first; its acc bank is the one
                # J+1 reuses first.
                for h in range(2):
                    emit_division(acc[Jp], aT[Jp], h)
                jobs.extend(
                    (lambda a=aT[Jp], Jx=Jp, sb=s: emit_outproj(a, Jx, sb))
                    for s in range(QB // KT)
                )

        for i, (J, t) in enumerate(units):
            if t == 0:
                acc[J] = [
                    acc_ps.tile([65, QB], F32, tag="acc", name=f"acc{J}_{_h}")
                    for _h in range(2)
                ]
                aT[J] = a_sb.tile([128, QB], F16, tag="aT", name=f"aT{J}")
                if J == 0:
                    emit_qkproj(0)
                    emit_vtiles(0)
                    emit_vtiles(2)
                if J + 1 < NQB:
                    jobs.append(lambda Jn=J + 1: emit_qkproj(Jn))
                    jobs.append(lambda tt=4 * (J + 1): emit_vtiles(tt))
                    jobs.append(lambda tt=4 * (J + 1) + 2: emit_vtiles(tt))

            diag = t * KT - J * QB  # >= 0 on diagonal-straddling tiles
            lo = max(diag, 0)
            st = sp_ps.tile([128, 2, QB], F32, tag="sp")
            for h in range(2):
                # 64-deep row-tiled pair: head A rows 0:64, head B 64:128.
                nc.tensor.matmul(
                    st[:, h, lo:QB],
                    kT2[bass.ts(h, 64), bass.ts(t, KT)],
                    qT_r[bass.ts(h, 64), bass.ds(J * QB + lo, QB - lo)],
                    start=True,
                    stop=True,
                )
            pt = p_sb.tile([128, 2, QB], F16, tag="pp", name=f"pp{i}")
            nc.scalar.activation(
                pt[:, :, lo:QB], st[:, :, lo:QB], EXP,
                scale=scale, bias=bias_t[:],
            )
            if diag >= 0:
                for h in range(2):
                    nc.vector.tensor_mul(
                        pt[:, h, diag : diag + KT],
                        pt[:, h, diag : diag + KT],
                        cmask[:, diag // KT, diag : diag + KT],
                    )
            pts[i] = (pt, lo)
            if jobs and t % 2 == 1:
                jobs.pop(0)()
            if i % 2 == 1 and i >= 3:
                emit_pv(i - 3)
                emit_pv(i - 2)

        emit_pv(len(units) - 2)
        emit_pv(len(units) - 1)
        for fl in jobs:
            fl()

    nc.compile()
    return nc


def get_nc():
    if "nc" not in _NC_CACHE:
        _NC_CACHE["nc"] = _build()
    return _NC_CACHE["nc"]


def make_in_maps(x, w_qkv, w_out):
    x = np.asarray(x, dtype=np.float32)
    w_qkv = np.asarray(w_qkv, dtype=np.float32)
    w_out = np.asarray(w_out, dtype=np.float32)
    in_maps = []
    for c in range(8):
        b, g = divmod(c, 4)
        rows = slice(g * 128, (g + 1) * 128)
        in_maps.append(
            {
                "xT": np.ascontiguousarray(x[b].T.astype(np.float16)),
                "wqT": np.ascontiguousarray(w_qkv[rows, :].T.astype(np.float16)),
                "wkT": np.ascontiguousarray(
                    w_qkv[512 + g * 128 : 512 + (g + 1) * 128, :].T.astype(
                        np.float16
                    )
                ),
                "wvT": np.ascontiguousarray(
                    w_qkv[1024 + g * 128 : 1024 + (g + 1) * 128, :].T.astype(
                        np.float16
                    )
                ),
                "woT": np.ascontiguousarray(w_out[:, rows].T.astype(np.float16)),
            }
        )
    return in_maps


def combine_results(results):
    y = np.zeros((B, T, D), dtype=np.float32)
    for c, r in enumerate(results):
        y[c // 4] += r["y"].astype(np.float32)
    return y


def kernel(x, w_qkv, w_out, trace=False):
    _install_ntff_shim()
    from concourse.bass_utils import run_bass_kernel_spmd

    nc = get_nc()
    in_maps = make_in_maps(x, w_qkv, w_out)
    r = run_bass_kernel_spmd(nc, in_maps, core_ids=list(range(8)), trace=trace)
    y = combine_results(r.results)
    if trace:
        return y, r
    return y
